# revision 4
# baseline (speedup 1.0000x reference)
"""Trainium2 Bass kernel for nn_GAT_39427799777563 (GAT message passing).

Layout: (n,k) pairs on SBUF partitions ("nk-layout"). A tile covers 128
items = 32 blocks; block t holds items 4t..4t+3: partition p = 32*c + k
is (item 4t+c, neighbor k). ent/wr are host-swizzled to fp16 so each
partition's tile data is one contiguous 6400 B DRAM row.

Per tile:
    we   = ent * wr                  fp16, DVE 2x packed mode
    e    = sum_d we[t-block]         fp32, DVE strided reduce (+ACT accum share)
    elr  = leaky_relu(e)             DVE STT
    ex   = exp(elr)                  ACT
    p    = ex * adj                  DVE (fp32, no overflow)
    Z    = per-item sum_k p          PE: mask4^T . p   -> [4, 32] PSUM
    Zb   = Z broadcast to (n,k)      PE: mask4T^T . Zt -> [128, 32] PSUM
    att  = p / Zb  (cast fp16)       DVE reciprocal + mul; att in [0,1]
    A    = blockdiag(att)            DVE: att[p,t] * mask4[p,c] -> [128, 4t+c]
    h'^T = We^T . A  per block       PE: 32 matmuls [128,100]^T x [128,4]
                                     -> PSUM [100, 4t:4t+4]  (fp16 in, fp32 acc)
    hs   = h'^T as fp16              ACT PSUM->SBUF copy
    x    = hs^T . wt16               PE -> PSUM [128, 100] fp32
    out  = x + item + b              residual add on host

Normalizing attention BEFORE the weighted sum keeps everything in fp16
range (att <= 1); exp/Z stay fp32 so no max-subtraction is needed.
Host-verified numerics: rel err 3.2e-3 vs fp32 reference (tol 2e-2).

Sharding: pure data parallel over N across 8 cores; rows padded
40000 -> 40960 so every core runs 40 full 128-item tiles.
"""

from contextlib import ExitStack
import os as _os

import numpy as np

import concourse.bass as bass
import concourse.bacc as bacc
import concourse.mybir as mybir
import concourse.tile as tile

F32 = mybir.dt.float32
F16 = mybir.dt.float16
ALPHA = 0.2

N, K, D = 40000, 32, 100
N_CORES = 8
P = 128
TB = P // K            # items per block = 4
NB = P // TB           # blocks per tile = 32
F = NB * D             # 3200 free elems per partition per tensor
M_DVE = int(_os.environ.get("GAT2_M_DVE", "24"))  # blocks e-summed on DVE
STORE_CHUNK = 8


def build(n_tiles: int, repeats: int = 1, mode: str = "full"):
    rows = n_tiles * P
    nc = bacc.Bacc("TRN2", target_bir_lowering=False, debug=False,
                   num_devices=N_CORES)

    ent_d = nc.dram_tensor("ent", [rows, F], F16, kind="ExternalInput")
    wr_d = nc.dram_tensor("wr", [rows, F], F16, kind="ExternalInput")
    adj_d = nc.dram_tensor("adjf", [P, n_tiles * NB], F32, kind="ExternalInput")
    wt_d = nc.dram_tensor("wt", [D, D], F16, kind="ExternalInput")
    m4_d = nc.dram_tensor("m4", [P, TB], F32, kind="ExternalInput")
    m4h_d = nc.dram_tensor("m4h", [P, TB], F16, kind="ExternalInput")
    m4t_d = nc.dram_tensor("m4t", [TB, P], F32, kind="ExternalInput")
    out_d = nc.dram_tensor("out", [P, n_tiles * D], F32, kind="ExternalOutput")

    AF = mybir.ActivationFunctionType
    AL = mybir.AluOpType
    AX = mybir.AxisListType

    with tile.TileContext(nc) as tc, ExitStack() as ctx:
        const = ctx.enter_context(tc.tile_pool(name="const", bufs=1))
        big = ctx.enter_context(tc.tile_pool(name="big", bufs=3))
        wep = ctx.enter_context(tc.tile_pool(name="wep", bufs=4))
        small = ctx.enter_context(tc.tile_pool(name="small", bufs=6))
        psum = ctx.enter_context(tc.tile_pool(name="psum", bufs=2, space="PSUM"))
        psum_ht = ctx.enter_context(tc.tile_pool(name="psum_ht", bufs=3,
                                                 space="PSUM"))

        adjf = const.tile([P, n_tiles * NB], F32)
        wt16 = const.tile([D, D], F16)
        m4 = const.tile([P, TB], F32)
        m4h = const.tile([P, TB], F16)
        m4t = const.tile([TB, P], F32)
        out_all = const.tile([P, n_tiles * D], F32)
        nc.sync.dma_start(adjf[:], adj_d[:])
        nc.sync.dma_start(wt16[:], wt_d[:])
        nc.sync.dma_start(m4[:], m4_d[:])
        nc.sync.dma_start(m4h[:], m4h_d[:])
        nc.sync.dma_start(m4t[:], m4t_d[:])

        if mode == "nodma":
            ent_r = const.tile([P, F], F16)
            wr_r = const.tile([P, F], F16)
            nc.sync.dma_start(ent_r[:], ent_d[:P, :])
            nc.sync.dma_start(wr_r[:], wr_d[:P, :])

        def phase1(g):
            """DMA + big DVE passes + softmax front + Z matmul."""
            rsl = slice(g * P, (g + 1) * P)
            if mode == "nodma":
                ent_t, wr_t = ent_r, wr_r
            else:
                ent_t = big.tile([P, F], F16, tag="ent")
                nc.sync.dma_start(ent_t[:], ent_d[rsl, :])
                wr_t = big.tile([P, F], F16, tag="wr")
                nc.sync.dma_start(wr_t[:], wr_d[rsl, :])

            if mode == "dma":
                nc.vector.tensor_copy(out_all[:, g * D:(g + 1) * D],
                                      ent_t[:, :D])
                return None

            # We = ent * wr (fp16, 2x packed)
            we = wep.tile([P, F], F16, tag="we")
            nc.vector.tensor_mul(we[:], ent_t[:], wr_t[:])

            # e[p, t] = sum_d we[p, t, d] (fp32, DVE/ACT split)
            e = small.tile([P, NB], F32, tag="e")
            if M_DVE > 0:
                nc.vector.tensor_reduce(
                    e[:, :M_DVE],
                    we[:, :M_DVE * D].rearrange("p (t d) -> p t d", t=M_DVE),
                    axis=AX.X, op=AL.add,
                )
            if M_DVE < NB:
                scr = small.tile([P, D], F16, tag="scr")
                for t in range(M_DVE, NB):
                    tsl = slice(t * D, (t + 1) * D)
                    nc.scalar.activation(scr[:], we[:, tsl], AF.Copy,
                                         accum_out=e[:, t:t + 1])

            if mode == "nosoft":
                nc.vector.tensor_copy(out_all[:, g * D:(g + 1) * D], we[:, :D])
                return None

            # leaky relu
            elr = small.tile([P, NB], F32, tag="elr")
            nc.vector.scalar_tensor_tensor(elr[:], e[:], ALPHA, e[:],
                                           op0=AL.mult, op1=AL.max)
            # exp (ACT)
            ex = small.tile([P, NB], F32, tag="ex")
            nc.scalar.activation(ex[:], elr[:], AF.Exp)
            # mask
            pm = small.tile([P, NB], F32, tag="pm")
            nc.vector.tensor_mul(pm[:], ex[:], adjf[:, g * NB:(g + 1) * NB])

            # Z[c, t] = per-item sums via PE (zzb tile: z cols 0:NB,
            # zb cols NB:2NB)
            zzb = psum.tile([P, 2 * NB], F32, tag="zzb")
            nc.tensor.matmul(zzb[:TB, :NB], m4[:], pm[:], start=True, stop=True)
            return {"we": we, "pm": pm, "zzb": zzb}

        def phase2a(g, st):
            """Normalize + block-diag A + PE weighted sum into ht PSUM."""
            if st is None:
                return None
            we, pm, zzb = st["we"], st["pm"], st["zzb"]

            # broadcast Z to partitions via PE, then normalize
            zt = small.tile([TB, NB], F32, tag="zt")
            nc.scalar.copy(zt[:], zzb[:TB, :NB])
            nc.tensor.matmul(zzb[:, NB:], m4t[:], zt[:], start=True, stop=True)

            rz = small.tile([P, NB], F32, tag="rz")
            nc.vector.reciprocal(rz[:], zzb[:, NB:])
            att = small.tile([P, NB], F16, tag="att")
            nc.vector.tensor_mul(att[:], pm[:], rz[:])

            # A[p, 4t+c] = att[p, t] * mask4[p, c]  (block-diagonal)
            A = small.tile([P, P], F16, tag="A")
            nc.vector.tensor_mul(
                A[:].rearrange("p (t c) -> p t c", c=TB),
                att[:].unsqueeze(-1).broadcast_to([P, NB, TB]),
                m4h[:].unsqueeze(1).broadcast_to([P, NB, TB]),
            )

            if mode == "nope":
                nc.vector.tensor_copy(out_all[:, g * D:(g + 1) * D], A[:, :D])
                return None

            # h'^T blocks: [100, 4t:4t+4] = we_block^T . A_block
            ht_ps = psum_ht.tile([D, P], F32, tag="ht")
            for t in range(NB):
                nc.tensor.matmul(ht_ps[:, t * TB:(t + 1) * TB],
                                 we[:, t * D:(t + 1) * D],
                                 A[:, t * TB:(t + 1) * TB],
                                 start=True, stop=True)
            return ht_ps

        def phase2b(g, ht_ps):
            """Deferred tail (runs 2 tiles later so no engine queue ever
            waits on a fresh PE result): hs copy, final matmul, out copy."""
            if ht_ps is not None:
                hs = small.tile([D, P], F16, tag="hs")
                nc.vector.tensor_copy(hs[:], ht_ps[:])
                # x = hs^T @ wt16 (residual + bias added on host)
                x_ps = psum.tile([P, D], F32, tag="x")
                nc.tensor.matmul(x_ps[:], hs[:], wt16[:], start=True, stop=True)
                nc.vector.tensor_copy(out_all[:, g * D:(g + 1) * D], x_ps[:])
            if (g + 1) % STORE_CHUNK == 0:
                csl = slice((g + 1 - STORE_CHUNK) * D, (g + 1) * D)
                nc.sync.dma_start(out_d[:, csl], out_all[:, csl])

        DELAY = 2

        def body():
            pend = []  # [(g, ht_ps), ...] awaiting phase2b
            for g in range(n_tiles):
                st = phase1(g)
                if len(pend) >= DELAY:
                    phase2b(*pend.pop(0))
                pend.append((g, phase2a(g, st)))
            for item in pend:
                phase2b(*item)

            rem = n_tiles % STORE_CHUNK
            if rem:
                csl = slice((n_tiles - rem) * D, n_tiles * D)
                nc.sync.dma_start(out_d[:, csl], out_all[:, csl])

        if repeats > 1:
            with tc.For_i(0, repeats, 1):
                body()
        else:
            body()

    nc.compile()
    return nc


def _shard_host(item_embs, entity_embs, w_r, adj, W_out, b_out, n_tiles):
    rows = n_tiles * P
    n_pad = N_CORES * rows

    ent = np.asarray(entity_embs, np.float16).reshape(N, K, D)
    wr = np.asarray(w_r, np.float16).reshape(N, K, D)
    adjf = np.asarray(adj).astype(np.float32)

    pad = n_pad - N
    ent = np.pad(ent, ((0, pad), (0, 0), (0, 0)))
    wr = np.pad(wr, ((0, pad), (0, 0), (0, 0)))
    adjf = np.pad(adjf, ((0, pad), (0, 0)), constant_values=1.0)

    wt = np.asarray(W_out, np.float32).T.astype(np.float16)
    wt = np.ascontiguousarray(wt)
    # mask4[p, c] = 1 if p // 32 == c   (item-slot indicator within a block)
    m4 = np.zeros((P, TB), np.float32)
    for c in range(TB):
        m4[c * K:(c + 1) * K, c] = 1.0
    m4h = m4.astype(np.float16)
    m4t = np.ascontiguousarray(m4.T)

    def nk_swizzle(a, rs):  # [rows, K, D] -> [rows, F] nk-layout
        # tile g, partition 32c+k, block t, d  <-  item g*128 + 4t + c, k, d
        x = a[rs].reshape(n_tiles, NB, TB, K, D)          # [g, t, c, k, d]
        x = x.transpose(0, 2, 3, 1, 4)                    # [g, c, k, t, d]
        return np.ascontiguousarray(x.reshape(rows, F))

    in_maps = []
    for c in range(N_CORES):
        rs = slice(c * rows, (c + 1) * rows)
        a = adjf[rs].reshape(n_tiles, NB, TB, K)          # [g, t, c, k]
        a_sw = np.ascontiguousarray(
            a.transpose(2, 3, 0, 1).reshape(P, n_tiles * NB))
        in_maps.append({
            "ent": nk_swizzle(ent, rs),
            "wr": nk_swizzle(wr, rs),
            "adjf": a_sw,
            "wt": wt,
            "m4": m4,
            "m4h": m4h,
            "m4t": m4t,
        })
    return in_maps


def _unshard_host(results, n_tiles):
    rows = n_tiles * P
    outs = []
    for c in range(N_CORES):
        o = results[c]["out"]
        outs.append(o.reshape(P, n_tiles, D).transpose(1, 0, 2).reshape(rows, D))
    return np.concatenate(outs)[:N]


_N_TILES_FULL = 40


def kernel(item_embs, entity_embs, w_r, adj, W_out, b_out):
    from concourse.bass_utils import run_bass_kernel_spmd

    nc = build(_N_TILES_FULL)
    in_maps = _shard_host(item_embs, entity_embs, w_r, adj, W_out, b_out,
                          _N_TILES_FULL)
    res = run_bass_kernel_spmd(nc, in_maps, core_ids=list(range(N_CORES)))
    x = _unshard_host(res.results, _N_TILES_FULL)
    return (x + np.asarray(item_embs, np.float32)
            + np.asarray(b_out, np.float32)).astype(np.float32)


# revision 8
# speedup vs baseline: 1.2654x; 1.2654x over previous
"""Trainium2 Bass kernel for nn_GAT_39427799777563 (GAT message passing).

Layout: (n,k) pairs on SBUF partitions ("nk-layout"). A tile covers 128
items = 32 blocks; block t holds items 4t..4t+3: partition p = 32*c + k
is (item 4t+c, neighbor k). ent/wr are host-swizzled to fp16 so each
partition's tile data is one contiguous 6400 B DRAM row.

Per tile:
    we   = ent * wr                  fp16, DVE 2x packed mode
    e    = sum_d we[t-block]         fp32, DVE strided reduce (+ACT accum share)
    elr  = leaky_relu(e)             DVE STT
    ex   = exp(elr)                  ACT
    p    = ex * adj                  DVE (fp32, no overflow)
    Z    = per-item sum_k p          PE: mask4^T . p   -> [4, 32] PSUM
    Zb   = Z broadcast to (n,k)      PE: mask4T^T . Zt -> [128, 32] PSUM
    att  = p / Zb  (cast fp16)       DVE reciprocal + mul; att in [0,1]
    A    = blockdiag(att)            DVE: att[p,t] * mask4[p,c] -> [128, 4t+c]
    h'^T = We^T . A  per block       PE: 32 matmuls [128,100]^T x [128,4]
                                     -> PSUM [100, 4t:4t+4]  (fp16 in, fp32 acc)
    hs   = h'^T as fp16              ACT PSUM->SBUF copy
    x    = hs^T . wt16               PE -> PSUM [128, 100] fp32
    out  = x + item + b              residual add on host

Normalizing attention BEFORE the weighted sum keeps everything in fp16
range (att <= 1); exp/Z stay fp32 so no max-subtraction is needed.
Host-verified numerics: rel err 3.2e-3 vs fp32 reference (tol 2e-2).

Sharding: pure data parallel over N across 8 cores; rows padded
40000 -> 40960 so every core runs 40 full 128-item tiles.
"""

from contextlib import ExitStack
import os as _os

import numpy as np

import concourse.bass as bass
import concourse.bacc as bacc
import concourse.mybir as mybir
import concourse.tile as tile

F32 = mybir.dt.float32
F16 = mybir.dt.float16
ALPHA = 0.2

N, K, D = 40000, 32, 100
N_CORES = 8
P = 128
TB = P // K            # items per block = 4
NB = P // TB           # blocks per tile = 32
F = NB * D             # 3200 free elems per partition per tensor
M_DVE = int(_os.environ.get("GAT2_M_DVE", "24"))  # blocks e-summed on DVE
STORE_CHUNK = 8


def build(n_tiles: int, repeats: int = 1, mode: str = "full"):
    rows = n_tiles * P
    nc = bacc.Bacc("TRN2", target_bir_lowering=False, debug=False,
                   num_devices=N_CORES)

    ent_d = nc.dram_tensor("ent", [rows, F], F16, kind="ExternalInput")
    wr_d = nc.dram_tensor("wr", [rows, F], F16, kind="ExternalInput")
    adj_d = nc.dram_tensor("adjf", [P, n_tiles * NB], F32, kind="ExternalInput")
    wt_d = nc.dram_tensor("wt", [D, D], F16, kind="ExternalInput")
    m4_d = nc.dram_tensor("m4", [P, TB], F32, kind="ExternalInput")
    m4h_d = nc.dram_tensor("m4h", [P, TB], F16, kind="ExternalInput")
    m4t_d = nc.dram_tensor("m4t", [TB, P], F32, kind="ExternalInput")
    out_d = nc.dram_tensor("out", [P, n_tiles * D], F32, kind="ExternalOutput")

    AF = mybir.ActivationFunctionType
    AL = mybir.AluOpType
    AX = mybir.AxisListType

    with tile.TileContext(nc) as tc, ExitStack() as ctx:
        const = ctx.enter_context(tc.tile_pool(name="const", bufs=1))
        big = ctx.enter_context(tc.tile_pool(name="big", bufs=3))
        wep = ctx.enter_context(tc.tile_pool(name="wep", bufs=4))
        small = ctx.enter_context(tc.tile_pool(name="small", bufs=6))
        psum = ctx.enter_context(tc.tile_pool(name="psum", bufs=2, space="PSUM"))
        psum_ht = ctx.enter_context(tc.tile_pool(name="psum_ht", bufs=3,
                                                 space="PSUM"))

        adjf = const.tile([P, n_tiles * NB], F32)
        wt16 = const.tile([D, D], F16)
        m4 = const.tile([P, TB], F32)
        m4h = const.tile([P, TB], F16)
        m4t = const.tile([TB, P], F32)
        out_all = const.tile([P, n_tiles * D], F32)
        nc.sync.dma_start(adjf[:], adj_d[:])
        nc.sync.dma_start(wt16[:], wt_d[:])
        nc.sync.dma_start(m4[:], m4_d[:])
        nc.sync.dma_start(m4h[:], m4h_d[:])
        nc.sync.dma_start(m4t[:], m4t_d[:])

        if mode == "nodma":
            ent_r = const.tile([P, F], F16)
            wr_r = const.tile([P, F], F16)
            nc.sync.dma_start(ent_r[:], ent_d[:P, :])
            nc.sync.dma_start(wr_r[:], wr_d[:P, :])

        def phase1(g):
            """DMA + big DVE passes + softmax front + Z matmul."""
            rsl = slice(g * P, (g + 1) * P)
            if mode == "nodma":
                ent_t, wr_t = ent_r, wr_r
            else:
                ent_t = big.tile([P, F], F16, tag="ent")
                nc.sync.dma_start(ent_t[:], ent_d[rsl, :])
                wr_t = big.tile([P, F], F16, tag="wr")
                nc.sync.dma_start(wr_t[:], wr_d[rsl, :])

            if mode == "dma":
                nc.vector.tensor_copy(out_all[:, g * D:(g + 1) * D],
                                      ent_t[:, :D])
                return None

            # We = ent * wr (fp16, 2x packed)
            we = wep.tile([P, F], F16, tag="we")
            nc.vector.tensor_mul(we[:], ent_t[:], wr_t[:])

            # e[p, t] = sum_d we[p, t, d] (fp32, DVE/ACT split)
            e = small.tile([P, NB], F32, tag="e")
            if M_DVE > 0:
                nc.vector.tensor_reduce(
                    e[:, :M_DVE],
                    we[:, :M_DVE * D].rearrange("p (t d) -> p t d", t=M_DVE),
                    axis=AX.X, op=AL.add,
                )
            if M_DVE < NB:
                scr = small.tile([P, D], F16, tag="scr")
                for t in range(M_DVE, NB):
                    tsl = slice(t * D, (t + 1) * D)
                    nc.scalar.activation(scr[:], we[:, tsl], AF.Copy,
                                         accum_out=e[:, t:t + 1])

            if mode == "nosoft":
                nc.vector.tensor_copy(out_all[:, g * D:(g + 1) * D], we[:, :D])
                return None

            # leaky relu
            elr = small.tile([P, NB], F32, tag="elr")
            nc.vector.scalar_tensor_tensor(elr[:], e[:], ALPHA, e[:],
                                           op0=AL.mult, op1=AL.max)
            # exp (ACT)
            ex = small.tile([P, NB], F32, tag="ex")
            nc.scalar.activation(ex[:], elr[:], AF.Exp)
            # mask
            pm = small.tile([P, NB], F32, tag="pm")
            nc.vector.tensor_mul(pm[:], ex[:], adjf[:, g * NB:(g + 1) * NB])

            # Z[c, t] = per-item sums via PE (zzb tile: z cols 0:NB,
            # zb cols NB:2NB)
            zzb = psum.tile([P, 2 * NB], F32, tag="zzb")
            nc.tensor.matmul(zzb[:TB, :NB], m4[:], pm[:], start=True, stop=True)
            return {"we": we, "pm": pm, "zzb": zzb}

        def phase2a(g, st):
            """Normalize + block-diag A + PE weighted sum into ht PSUM."""
            if st is None:
                return None
            we, pm, zzb = st["we"], st["pm"], st["zzb"]

            # broadcast Z to partitions via PE, then normalize
            zt = small.tile([TB, NB], F32, tag="zt")
            nc.scalar.copy(zt[:], zzb[:TB, :NB])
            nc.tensor.matmul(zzb[:, NB:], m4t[:], zt[:], start=True, stop=True)

            rz = small.tile([P, NB], F32, tag="rz")
            nc.vector.reciprocal(rz[:], zzb[:, NB:])
            att = small.tile([P, NB], F16, tag="att")
            nc.vector.tensor_mul(att[:], pm[:], rz[:])

            # A[p, 4t+c] = att[p, t] * mask4[p, c]  (block-diagonal)
            A = small.tile([P, P], F16, tag="A")
            nc.vector.tensor_mul(
                A[:].rearrange("p (t c) -> p t c", c=TB),
                att[:].unsqueeze(-1).broadcast_to([P, NB, TB]),
                m4h[:].unsqueeze(1).broadcast_to([P, NB, TB]),
            )

            if mode == "nope":
                nc.vector.tensor_copy(out_all[:, g * D:(g + 1) * D], A[:, :D])
                return None

            # h'^T blocks: [100, 4t:4t+4] = we_block^T . A_block
            ht_ps = psum_ht.tile([D, P], F32, tag="ht")
            for t in range(NB):
                nc.tensor.matmul(ht_ps[:, t * TB:(t + 1) * TB],
                                 we[:, t * D:(t + 1) * D],
                                 A[:, t * TB:(t + 1) * TB],
                                 start=True, stop=True)
            return ht_ps

        def phase2b(g, ht_ps):
            """Deferred tail (runs 2 tiles later so no engine queue ever
            waits on a fresh PE result): hs copy, final matmul, out copy."""
            if ht_ps is not None:
                hs = small.tile([D, P], F16, tag="hs")
                nc.vector.tensor_copy(hs[:], ht_ps[:])
                # x = hs^T @ wt16 (residual + bias added on host)
                x_ps = psum.tile([P, D], F32, tag="x")
                nc.tensor.matmul(x_ps[:], hs[:], wt16[:], start=True, stop=True)
                nc.vector.tensor_copy(out_all[:, g * D:(g + 1) * D], x_ps[:])
            if (g + 1) % STORE_CHUNK == 0:
                csl = slice((g + 1 - STORE_CHUNK) * D, (g + 1) * D)
                nc.sync.dma_start(out_d[:, csl], out_all[:, csl])

        DELAY = 2

        def body():
            pend = []  # [(g, ht_ps), ...] awaiting phase2b
            for g in range(n_tiles):
                st = phase1(g)
                if len(pend) >= DELAY:
                    phase2b(*pend.pop(0))
                pend.append((g, phase2a(g, st)))
            for item in pend:
                phase2b(*item)

            rem = n_tiles % STORE_CHUNK
            if rem:
                csl = slice((n_tiles - rem) * D, n_tiles * D)
                nc.sync.dma_start(out_d[:, csl], out_all[:, csl])

        if repeats > 1:
            with tc.For_i(0, repeats, 1):
                body()
        else:
            body()

    nc.compile()
    return nc


def _shard_host(item_embs, entity_embs, w_r, adj, W_out, b_out, n_tiles):
    rows = n_tiles * P
    n_pad = N_CORES * rows

    ent = np.asarray(entity_embs, np.float16).reshape(N, K, D)
    wr = np.asarray(w_r, np.float16).reshape(N, K, D)
    adjf = np.asarray(adj).astype(np.float32)

    pad = n_pad - N
    ent = np.pad(ent, ((0, pad), (0, 0), (0, 0)))
    wr = np.pad(wr, ((0, pad), (0, 0), (0, 0)))
    adjf = np.pad(adjf, ((0, pad), (0, 0)), constant_values=1.0)

    wt = np.asarray(W_out, np.float32).T.astype(np.float16)
    wt = np.ascontiguousarray(wt)
    # mask4[p, c] = 1 if p // 32 == c   (item-slot indicator within a block)
    m4 = np.zeros((P, TB), np.float32)
    for c in range(TB):
        m4[c * K:(c + 1) * K, c] = 1.0
    m4h = m4.astype(np.float16)
    m4t = np.ascontiguousarray(m4.T)

    def nk_swizzle(a, rs):  # [rows, K, D] -> [rows, F] nk-layout
        # tile g, partition 32c+k, block t, d  <-  item g*128 + 4t + c, k, d
        x = a[rs].reshape(n_tiles, NB, TB, K, D)          # [g, t, c, k, d]
        x = x.transpose(0, 2, 3, 1, 4)                    # [g, c, k, t, d]
        return np.ascontiguousarray(x.reshape(rows, F))

    in_maps = []
    for c in range(N_CORES):
        rs = slice(c * rows, (c + 1) * rows)
        a = adjf[rs].reshape(n_tiles, NB, TB, K)          # [g, t, c, k]
        a_sw = np.ascontiguousarray(
            a.transpose(2, 3, 0, 1).reshape(P, n_tiles * NB))
        in_maps.append({
            "ent": nk_swizzle(ent, rs),
            "wr": nk_swizzle(wr, rs),
            "adjf": a_sw,
            "wt": wt,
            "m4": m4,
            "m4h": m4h,
            "m4t": m4t,
        })
    return in_maps


def _unshard_host(results, n_tiles):
    rows = n_tiles * P
    outs = []
    for c in range(N_CORES):
        o = results[c]["out"]
        outs.append(o.reshape(P, n_tiles, D).transpose(1, 0, 2).reshape(rows, D))
    return np.concatenate(outs)[:N]


_N_TILES_FULL = 40


def kernel(item_embs, entity_embs, w_r, adj, W_out, b_out):
    from concourse.bass_utils import run_bass_kernel_spmd

    nc = build(_N_TILES_FULL)
    in_maps = _shard_host(item_embs, entity_embs, w_r, adj, W_out, b_out,
                          _N_TILES_FULL)
    res = run_bass_kernel_spmd(nc, in_maps, core_ids=list(range(N_CORES)))
    x = _unshard_host(res.results, _N_TILES_FULL)
    return (x + np.asarray(item_embs, np.float32)
            + np.asarray(b_out, np.float32)).astype(np.float32)


# revision 9
# speedup vs baseline: 1.3794x; 1.0901x over previous
"""Trainium2 Bass kernel for nn_GAT_39427799777563 (GAT message passing).

Layout: (n,k) pairs on SBUF partitions ("nk-layout"). A tile covers 128
items = 32 blocks; block t holds items 4t..4t+3: partition p = 32*c + k
is (item 4t+c, neighbor k). ent/wr are host-swizzled to fp16 so each
partition's tile data is one contiguous 6400 B DRAM row.

Per tile:
    we   = ent * wr                  fp16, DVE 2x packed mode
    e    = sum_d we[t-block]         fp32, DVE strided reduce (+ACT accum share)
    elr  = leaky_relu(e)             DVE STT
    ex   = exp(elr)                  ACT
    p    = ex * adj                  DVE (fp32, no overflow)
    Z    = per-item sum_k p          PE: mask4^T . p   -> [4, 32] PSUM
    Zb   = Z broadcast to (n,k)      PE: mask4T^T . Zt -> [128, 32] PSUM
    att  = p / Zb  (cast fp16)       DVE reciprocal + mul; att in [0,1]
    A    = blockdiag(att)            DVE: att[p,t] * mask4[p,c] -> [128, 4t+c]
    h'^T = We^T . A  per block       PE: 32 matmuls [128,100]^T x [128,4]
                                     -> PSUM [100, 4t:4t+4]  (fp16 in, fp32 acc)
    hs   = h'^T as fp16              ACT PSUM->SBUF copy
    x    = hs^T . wt16               PE -> PSUM [128, 100] fp32
    out  = x + item + b              residual add on host

Normalizing attention BEFORE the weighted sum keeps everything in fp16
range (att <= 1); exp/Z stay fp32 so no max-subtraction is needed.
Host-verified numerics: rel err 3.2e-3 vs fp32 reference (tol 2e-2).

Sharding: pure data parallel over N across 8 cores; rows padded
40000 -> 40960 so every core runs 40 full 128-item tiles.
"""

from contextlib import ExitStack
import os as _os

import numpy as np

import concourse.bass as bass
import concourse.bacc as bacc
import concourse.mybir as mybir
import concourse.tile as tile

F32 = mybir.dt.float32
F16 = mybir.dt.float16
ALPHA = 0.2

N, K, D = 40000, 32, 100
N_CORES = 8
P = 128
TB = P // K            # items per block = 4
NB = P // TB           # blocks per tile = 32
F = NB * D             # 3200 free elems per partition per tensor
M_DVE = int(_os.environ.get("GAT2_M_DVE", "32"))  # blocks e-summed on DVE
STORE_CHUNK = 8


def build(n_tiles: int, repeats: int = 1, mode: str = "full"):
    rows = n_tiles * P
    nc = bacc.Bacc("TRN2", target_bir_lowering=False, debug=False,
                   num_devices=N_CORES)

    ent_d = nc.dram_tensor("ent", [rows, F], F16, kind="ExternalInput")
    wr_d = nc.dram_tensor("wr", [rows, F], F16, kind="ExternalInput")
    adj_d = nc.dram_tensor("adjf", [P, n_tiles * NB], F32, kind="ExternalInput")
    wt_d = nc.dram_tensor("wt", [D, D], F16, kind="ExternalInput")
    m4_d = nc.dram_tensor("m4", [P, TB], F32, kind="ExternalInput")
    m4h_d = nc.dram_tensor("m4h", [P, TB], F16, kind="ExternalInput")
    m4t_d = nc.dram_tensor("m4t", [TB, P], F32, kind="ExternalInput")
    out_d = nc.dram_tensor("out", [P, n_tiles * D], F32, kind="ExternalOutput")

    AF = mybir.ActivationFunctionType
    AL = mybir.AluOpType
    AX = mybir.AxisListType

    with tile.TileContext(nc) as tc, ExitStack() as ctx:
        const = ctx.enter_context(tc.tile_pool(name="const", bufs=1))
        big = ctx.enter_context(tc.tile_pool(name="big", bufs=3))
        wep = ctx.enter_context(tc.tile_pool(name="wep", bufs=4))
        small = ctx.enter_context(tc.tile_pool(name="small", bufs=6))
        psum = ctx.enter_context(tc.tile_pool(name="psum", bufs=2, space="PSUM"))
        psum_ht = ctx.enter_context(tc.tile_pool(name="psum_ht", bufs=3,
                                                 space="PSUM"))

        adjf = const.tile([P, n_tiles * NB], F32)
        wt16 = const.tile([D, D], F16)
        m4 = const.tile([P, TB], F32)
        m4h = const.tile([P, TB], F16)
        m4t = const.tile([TB, P], F32)
        out_all = const.tile([P, n_tiles * D], F32)
        nc.sync.dma_start(adjf[:], adj_d[:])
        nc.sync.dma_start(wt16[:], wt_d[:])
        nc.sync.dma_start(m4[:], m4_d[:])
        nc.sync.dma_start(m4h[:], m4h_d[:])
        nc.sync.dma_start(m4t[:], m4t_d[:])

        if mode == "nodma":
            ent_r = const.tile([P, F], F16)
            wr_r = const.tile([P, F], F16)
            nc.sync.dma_start(ent_r[:], ent_d[:P, :])
            nc.sync.dma_start(wr_r[:], wr_d[:P, :])

        def phase1(g):
            """DMA + big DVE passes + softmax front + Z matmul."""
            rsl = slice(g * P, (g + 1) * P)
            if mode == "nodma":
                ent_t, wr_t = ent_r, wr_r
            else:
                ent_t = big.tile([P, F], F16, tag="ent")
                nc.sync.dma_start(ent_t[:], ent_d[rsl, :])
                wr_t = big.tile([P, F], F16, tag="wr")
                nc.sync.dma_start(wr_t[:], wr_d[rsl, :])

            if mode == "dma":
                nc.vector.tensor_copy(out_all[:, g * D:(g + 1) * D],
                                      ent_t[:, :D])
                return None

            # We = ent * wr (fp16, 2x packed)
            we = wep.tile([P, F], F16, tag="we")
            nc.vector.tensor_mul(we[:], ent_t[:], wr_t[:])

            # e[p, t] = sum_d we[p, t, d] (fp32, DVE/ACT split)
            e = small.tile([P, NB], F32, tag="e")
            if M_DVE > 0:
                nc.vector.tensor_reduce(
                    e[:, :M_DVE],
                    we[:, :M_DVE * D].rearrange("p (t d) -> p t d", t=M_DVE),
                    axis=AX.X, op=AL.add,
                )
            if M_DVE < NB:
                scr = small.tile([P, D], F16, tag="scr")
                for t in range(M_DVE, NB):
                    tsl = slice(t * D, (t + 1) * D)
                    nc.scalar.activation(scr[:], we[:, tsl], AF.Copy,
                                         accum_out=e[:, t:t + 1])

            if mode == "nosoft":
                nc.vector.tensor_copy(out_all[:, g * D:(g + 1) * D], we[:, :D])
                return None

            # leaky relu
            elr = small.tile([P, NB], F32, tag="elr")
            nc.vector.scalar_tensor_tensor(elr[:], e[:], ALPHA, e[:],
                                           op0=AL.mult, op1=AL.max)
            # exp (ACT)
            ex = small.tile([P, NB], F32, tag="ex")
            nc.scalar.activation(ex[:], elr[:], AF.Exp)
            # mask
            pm = small.tile([P, NB], F32, tag="pm")
            nc.vector.tensor_mul(pm[:], ex[:], adjf[:, g * NB:(g + 1) * NB])

            # Z[c, t] = per-item sums via PE (zzb tile: z cols 0:NB,
            # zb cols NB:2NB)
            zzb = psum.tile([P, 2 * NB], F32, tag="zzb")
            nc.tensor.matmul(zzb[:TB, :NB], m4[:], pm[:], start=True, stop=True)
            return {"we": we, "pm": pm, "zzb": zzb}

        def phase2a(g, st):
            """Normalize + block-diag A + PE weighted sum into ht PSUM."""
            if st is None:
                return None
            we, pm, zzb = st["we"], st["pm"], st["zzb"]

            # broadcast Z to partitions via PE, then normalize
            zt = small.tile([TB, NB], F32, tag="zt")
            nc.scalar.copy(zt[:], zzb[:TB, :NB])
            nc.tensor.matmul(zzb[:, NB:], m4t[:], zt[:], start=True, stop=True)

            rz = small.tile([P, NB], F32, tag="rz")
            nc.vector.reciprocal(rz[:], zzb[:, NB:])
            att = small.tile([P, NB], F16, tag="att")
            nc.vector.tensor_mul(att[:], pm[:], rz[:])

            # A[p, 4t+c] = att[p, t] * mask4[p, c]  (block-diagonal)
            A = small.tile([P, P], F16, tag="A")
            nc.vector.tensor_mul(
                A[:].rearrange("p (t c) -> p t c", c=TB),
                att[:].unsqueeze(-1).broadcast_to([P, NB, TB]),
                m4h[:].unsqueeze(1).broadcast_to([P, NB, TB]),
            )

            if mode == "nope":
                nc.vector.tensor_copy(out_all[:, g * D:(g + 1) * D], A[:, :D])
                return None

            # h'^T blocks: [100, 4t:4t+4] = we_block^T . A_block
            ht_ps = psum_ht.tile([D, P], F32, tag="ht")
            for t in range(NB):
                nc.tensor.matmul(ht_ps[:, t * TB:(t + 1) * TB],
                                 we[:, t * D:(t + 1) * D],
                                 A[:, t * TB:(t + 1) * TB],
                                 start=True, stop=True)
            return ht_ps

        def phase2b(g, ht_ps):
            """Deferred tail (runs 2 tiles later so no engine queue ever
            waits on a fresh PE result): hs copy, final matmul, out copy."""
            if ht_ps is not None:
                hs = small.tile([D, P], F16, tag="hs")
                nc.vector.tensor_copy(hs[:], ht_ps[:])
                # x = hs^T @ wt16 (residual + bias added on host)
                x_ps = psum.tile([P, D], F32, tag="x")
                nc.tensor.matmul(x_ps[:], hs[:], wt16[:], start=True, stop=True)
                nc.vector.tensor_copy(out_all[:, g * D:(g + 1) * D], x_ps[:])
            if (g + 1) % STORE_CHUNK == 0:
                csl = slice((g + 1 - STORE_CHUNK) * D, (g + 1) * D)
                nc.sync.dma_start(out_d[:, csl], out_all[:, csl])

        DELAY = 2

        def body():
            pend = []  # [(g, ht_ps), ...] awaiting phase2b
            for g in range(n_tiles):
                st = phase1(g)
                if len(pend) >= DELAY:
                    phase2b(*pend.pop(0))
                pend.append((g, phase2a(g, st)))
            for item in pend:
                phase2b(*item)

            rem = n_tiles % STORE_CHUNK
            if rem:
                csl = slice((n_tiles - rem) * D, n_tiles * D)
                nc.sync.dma_start(out_d[:, csl], out_all[:, csl])

        if repeats > 1:
            with tc.For_i(0, repeats, 1):
                body()
        else:
            body()

    nc.compile()
    return nc


def _shard_host(item_embs, entity_embs, w_r, adj, W_out, b_out, n_tiles):
    rows = n_tiles * P
    n_pad = N_CORES * rows

    ent = np.asarray(entity_embs, np.float16).reshape(N, K, D)
    wr = np.asarray(w_r, np.float16).reshape(N, K, D)
    adjf = np.asarray(adj).astype(np.float32)

    pad = n_pad - N
    ent = np.pad(ent, ((0, pad), (0, 0), (0, 0)))
    wr = np.pad(wr, ((0, pad), (0, 0), (0, 0)))
    adjf = np.pad(adjf, ((0, pad), (0, 0)), constant_values=1.0)

    wt = np.asarray(W_out, np.float32).T.astype(np.float16)
    wt = np.ascontiguousarray(wt)
    # mask4[p, c] = 1 if p // 32 == c   (item-slot indicator within a block)
    m4 = np.zeros((P, TB), np.float32)
    for c in range(TB):
        m4[c * K:(c + 1) * K, c] = 1.0
    m4h = m4.astype(np.float16)
    m4t = np.ascontiguousarray(m4.T)

    def nk_swizzle(a, rs):  # [rows, K, D] -> [rows, F] nk-layout
        # tile g, partition 32c+k, block t, d  <-  item g*128 + 4t + c, k, d
        x = a[rs].reshape(n_tiles, NB, TB, K, D)          # [g, t, c, k, d]
        x = x.transpose(0, 2, 3, 1, 4)                    # [g, c, k, t, d]
        return np.ascontiguousarray(x.reshape(rows, F))

    in_maps = []
    for c in range(N_CORES):
        rs = slice(c * rows, (c + 1) * rows)
        a = adjf[rs].reshape(n_tiles, NB, TB, K)          # [g, t, c, k]
        a_sw = np.ascontiguousarray(
            a.transpose(2, 3, 0, 1).reshape(P, n_tiles * NB))
        in_maps.append({
            "ent": nk_swizzle(ent, rs),
            "wr": nk_swizzle(wr, rs),
            "adjf": a_sw,
            "wt": wt,
            "m4": m4,
            "m4h": m4h,
            "m4t": m4t,
        })
    return in_maps


def _unshard_host(results, n_tiles):
    rows = n_tiles * P
    outs = []
    for c in range(N_CORES):
        o = results[c]["out"]
        outs.append(o.reshape(P, n_tiles, D).transpose(1, 0, 2).reshape(rows, D))
    return np.concatenate(outs)[:N]


_N_TILES_FULL = 40


def kernel(item_embs, entity_embs, w_r, adj, W_out, b_out):
    from concourse.bass_utils import run_bass_kernel_spmd

    nc = build(_N_TILES_FULL)
    in_maps = _shard_host(item_embs, entity_embs, w_r, adj, W_out, b_out,
                          _N_TILES_FULL)
    res = run_bass_kernel_spmd(nc, in_maps, core_ids=list(range(N_CORES)))
    x = _unshard_host(res.results, _N_TILES_FULL)
    return (x + np.asarray(item_embs, np.float32)
            + np.asarray(b_out, np.float32)).astype(np.float32)
